# revision 33
# baseline (speedup 1.0000x reference)
"""Trainium2 Bass kernel for nn_BackwardReasonModel (gnn_message_passing).

Math reduction: fact_rel is all-ones so every row of fact_val equals
v = W.sum(axis=1) + b.  The two scatter-adds therefore produce
agg[s, :] = count[s] * v where count[s] = #occurrences of slot s in
batch_tails ++ batch_heads, and relu(count * v) = count * relu(v) since
count >= 0.  The kernel computes the 65536-bin histogram of the 1M
indices on-device via one-hot matmuls on the TensorEngine (256x256 bin
factorization, accumulated in PSUM), ReduceScatters the per-core partial
histograms across the 8 cores, and each core then writes its 8192-slot
slice of the output as count ⊗ relu(v).

Sharding: facts are split 8 ways (65536 facts -> 131072 head+tail
indices per core); output slots are split 8 ways by the ReduceScatter.
"""

import numpy as np
import ml_dtypes

import concourse.mybir as mybir
import concourse.tile as tile
import concourse.bacc as bacc
from concourse import library_config
from concourse.bass_utils import run_bass_kernel_spmd

NCORES = 8
BATCH = 32
MAX_LOCAL_ENTITY = 2048
NUM_FACT = 524288
HIDDEN = 128
N_SLOTS = BATCH * MAX_LOCAL_ENTITY          # 65536
FACTS_PER_CORE = NUM_FACT // NCORES         # 65536
N_IDX = 2 * FACTS_PER_CORE                  # 131072 indices per core
COLS = N_IDX // 128                         # 1024 columns of 128 indices
SLOTS_PER_CORE = N_SLOTS // NCORES          # 8192
OUT_GROUPS = SLOTS_PER_CORE // 128          # 64

F32 = mybir.dt.float32
BF16 = mybir.dt.bfloat16
I32 = mybir.dt.int32

# column blocking: per block of BLK columns, the first PHI hi-one-hots and
# the first PLO lo-one-hots are built by Pool local_scatter ops, the next
# AHI hi-one-hots by the scalar engine (Abs + Relu), the rest plus the
# remaining lo-one-hots by the DVE is_equal.
BLK = 12
PHI = 7
AHI = 3
PLO = 4


def build_kernel():
    nc = bacc.Bacc("TRN2", target_bir_lowering=False, debug=False,
                   num_devices=NCORES)

    tails = nc.dram_tensor("tails", [FACTS_PER_CORE], I32, kind="ExternalInput")
    heads = nc.dram_tensor("heads", [FACTS_PER_CORE], I32, kind="ExternalInput")
    w_in = nc.dram_tensor("W", [HIDDEN, HIDDEN], F32, kind="ExternalInput")
    b_in = nc.dram_tensor("b", [HIDDEN], F32, kind="ExternalInput")
    out = nc.dram_tensor("out", [SLOTS_PER_CORE, HIDDEN], F32,
                         kind="ExternalOutput")
    count_part = nc.dram_tensor("count_part", [N_SLOTS], BF16)
    count_rs = nc.dram_tensor("count_rs", [SLOTS_PER_CORE], BF16)
    iota_in = nc.dram_tensor("iota_bf", [128, 256], BF16, kind="ExternalInput")
    ident_in = nc.dram_tensor("ident", [128, 128], F32, kind="ExternalInput")
    offpat_in = nc.dram_tensor("offpat", [128, COLS], F32,
                               kind="ExternalInput")

    NBLK = COLS // BLK          # 85 full blocks
    TAIL0 = NBLK * BLK          # 1020; tail columns go to DVE

    with tile.TileContext(nc) as tc:
        with (
            tc.tile_pool(name="const", bufs=1) as cpool,
            tc.tile_pool(name="idx", bufs=1) as ipool,
            tc.tile_pool(name="oh", bufs=12) as ohpool,
            tc.tile_pool(name="poh", bufs=6) as pohpool,
            tc.tile_pool(name="outp", bufs=8) as opool,
            tc.tile_pool(name="psum", bufs=1, space="PSUM") as ppool,
        ):
            nc.gpsimd.load_library(library_config.local_scatter)

            # ---- Phase 0: constants + rv_row = relu(W.sum(1) + b) as a row
            iota_bf = cpool.tile([128, 256], BF16)
            nc.sync.dma_start(out=iota_bf[:], in_=iota_in.ap())
            ident = cpool.tile([128, 128], F32)
            nc.sync.dma_start(out=ident[:], in_=ident_in.ap())
            w_t = cpool.tile([128, 128], F32)
            nc.sync.dma_start(out=w_t[:], in_=w_in.ap())
            b_t = cpool.tile([128, 1], F32)
            nc.sync.dma_start(out=b_t[:], in_=b_in.ap().rearrange("(p o) -> p o", o=1))
            v_col = cpool.tile([128, 1], F32)
            nc.vector.reduce_sum(v_col[:], w_t[:], axis=mybir.AxisListType.X)
            v2_col = cpool.tile([128, 1], F32)
            nc.vector.tensor_tensor(out=v2_col[:], in0=v_col[:], in1=b_t[:],
                                    op=mybir.AluOpType.add)
            v_row_ps = ppool.tile([128, 128], F32)
            nc.tensor.transpose(out=v_row_ps[:],
                                in_=v2_col[:].to_broadcast([128, 128]),
                                identity=ident[:])
            rv_row = cpool.tile([128, 128], F32)
            nc.vector.tensor_scalar_max(rv_row[:], v_row_ps[:], 0.0)

            # ---- Phase 1: load indices, split into hi/lo planes ----
            idx = ipool.tile([128, COLS], I32)
            nc.sync.dma_start(
                out=idx[:, 0:COLS // 2],
                in_=tails.ap().rearrange("(p c) -> p c", p=128))
            nc.sync.dma_start(
                out=idx[:, COLS // 2:COLS],
                in_=heads.ap().rearrange("(p c) -> p c", p=128))
            hi_i = ipool.tile([128, COLS], I32)
            nc.vector.tensor_scalar(hi_i[:], idx[:], 8, None,
                                    op0=mybir.AluOpType.logical_shift_right)
            lo_i = ipool.tile([128, COLS], I32)
            nc.vector.tensor_scalar(lo_i[:], idx[:], 255, None,
                                    op0=mybir.AluOpType.bitwise_and)
            hi_f = ipool.tile([128, COLS], F32)
            nc.vector.tensor_copy(out=hi_f[:], in_=hi_i[:])
            lo_f = ipool.tile([128, COLS], F32)
            nc.vector.tensor_copy(out=lo_f[:], in_=lo_i[:])

            # Pool local_scatter index pairs: per block, 8 int16 idxs
            # [hi0, hi1+256, .., hi6+1536, -1] (offsets make them unique)
            offp = ipool.tile([128, COLS], F32)
            nc.sync.dma_start(out=offp[:], in_=offpat_in.ap())
            hioff_f = ipool.tile([128, COLS], F32)
            nc.vector.tensor_tensor(out=hioff_f[:], in0=hi_f[:], in1=offp[:],
                                    op=mybir.AluOpType.add)
            pidx = ipool.tile([128, 8 * NBLK], mybir.dt.int16)
            nc.vector.memset(pidx[:], -1)
            nc.vector.tensor_copy(
                out=pidx[:].rearrange("p (g b) -> p g b", b=8)[:, :, 0:PHI],
                in_=hioff_f[:, 0:TAIL0].rearrange(
                    "p (g b) -> p g b", b=BLK)[:, :, 0:PHI])
            looff_f = ipool.tile([128, COLS], F32)
            nc.vector.tensor_tensor(out=looff_f[:], in0=lo_f[:], in1=offp[:],
                                    op=mybir.AluOpType.add)
            pidx_lo = ipool.tile([128, 8 * NBLK], mybir.dt.int16)
            nc.vector.memset(pidx_lo[:], -1)
            nc.vector.tensor_copy(
                out=pidx_lo[:].rearrange("p (g b) -> p g b", b=8)[:, :, 0:PLO],
                in_=looff_f[:, 0:TAIL0].rearrange(
                    "p (g b) -> p g b", b=BLK)[:, :, 0:PLO])
            ones8 = cpool.tile([128, 8], BF16)
            nc.vector.memset(ones8[:], 1.0)

            # ---- Phase 2: histogram via one-hot matmuls ----
            psum0 = ppool.tile([128, 256], F32)
            psum1 = ppool.tile([128, 256], F32)

            def pool_lo(c):
                return BLK <= c < TAIL0 and c % BLK < PLO

            pt = None
            plt = None
            lo_cache = {}
            hi_cache = {}
            for c in range(COLS):
                r = c % BLK
                in_blk = BLK <= c < TAIL0
                if in_blk and r == 0:
                    g = c // BLK
                    pt = pohpool.tile([128, 256 * PHI], BF16)
                    nc.gpsimd.local_scatter(
                        pt[:], ones8[:], pidx[:, 8 * g:8 * g + 8],
                        channels=128, num_elems=256 * PHI, num_idxs=8)
                    plt = pohpool.tile([128, 256 * PLO], BF16)
                    nc.gpsimd.local_scatter(
                        plt[:], ones8[:], pidx_lo[:, 8 * g:8 * g + 8],
                        channels=128, num_elems=256 * PLO, num_idxs=8)
                if in_blk and r < PHI:
                    st0 = pt[:, 256 * r:256 * r + 128]
                    st1 = pt[:, 256 * r + 128:256 * (r + 1)]
                elif in_blk and r < PHI + AHI:
                    # hi one-hot on the scalar engine:
                    # t = |iota - hi|; onehot = relu(1 - t)
                    t = ohpool.tile([128, 256], BF16)
                    nc.scalar.activation(t[:], iota_bf[:],
                                         mybir.ActivationFunctionType.Abs,
                                         bias=hi_f[:, c:c + 1], scale=-1.0)
                    hi_oh = ohpool.tile([128, 256], BF16)
                    nc.scalar.activation(hi_oh[:], t[:],
                                         mybir.ActivationFunctionType.Relu,
                                         bias=1.0, scale=-1.0)
                    st0, st1 = hi_oh[:, 0:128], hi_oh[:, 128:256]
                elif c in hi_cache:
                    hw = hi_cache.pop(c)
                    st0, st1 = hw[0], hw[1]
                else:
                    nxt = c + 1
                    nxt_dve = (nxt < COLS and not (
                        BLK <= nxt < TAIL0 and nxt % BLK < PHI + AHI))
                    if nxt_dve:
                        hi2 = ohpool.tile([128, 512], BF16)
                        nc.vector.tensor_scalar(
                            hi2[:, 0:256], iota_bf[:], hi_f[:, c:c + 1], None,
                            op0=mybir.AluOpType.is_equal)
                        nc.vector.tensor_scalar(
                            hi2[:, 256:512], iota_bf[:], hi_f[:, nxt:nxt + 1],
                            None, op0=mybir.AluOpType.is_equal)
                        hi_cache[nxt] = (hi2[:, 256:384], hi2[:, 384:512])
                        st0, st1 = hi2[:, 0:128], hi2[:, 128:256]
                    else:
                        hi_oh = ohpool.tile([128, 256], BF16)
                        nc.vector.tensor_scalar(hi_oh[:], iota_bf[:],
                                                hi_f[:, c:c + 1], None,
                                                op0=mybir.AluOpType.is_equal)
                        st0, st1 = hi_oh[:, 0:128], hi_oh[:, 128:256]
                if pool_lo(c):
                    lo_mv = plt[:, 256 * r:256 * (r + 1)]
                elif c in lo_cache:
                    lo_mv = lo_cache.pop(c)
                else:
                    # pair two consecutive DVE-built lo one-hots in one tile
                    # to halve Tile bookkeeping on the vector engine
                    if c + 1 < COLS and not pool_lo(c + 1):
                        lo2 = ohpool.tile([128, 512], BF16)
                        nc.vector.tensor_scalar(
                            lo2[:, 0:256], iota_bf[:], lo_f[:, c:c + 1], None,
                            op0=mybir.AluOpType.is_equal)
                        nc.vector.tensor_scalar(
                            lo2[:, 256:512], iota_bf[:], lo_f[:, c + 1:c + 2],
                            None, op0=mybir.AluOpType.is_equal)
                        lo_cache[c + 1] = lo2[:, 256:512]
                        lo_mv = lo2[:, 0:256]
                    else:
                        lo_oh = ohpool.tile([128, 256], BF16)
                        nc.vector.tensor_scalar(lo_oh[:], iota_bf[:],
                                                lo_f[:, c:c + 1], None,
                                                op0=mybir.AluOpType.is_equal)
                        lo_mv = lo_oh[:]
                first = c == 0
                last = c == COLS - 1
                nc.tensor.matmul(out=psum0[:], lhsT=st0,
                                 rhs=lo_mv, start=first, stop=last)
                nc.tensor.matmul(out=psum1[:], lhsT=st1,
                                 rhs=lo_mv, start=first, stop=last)

            # ---- Phase 3: flatten per-core histogram to DRAM ----
            cnt0_sb = ipool.tile([128, 256], BF16)
            cnt1_sb = ipool.tile([128, 256], BF16)
            nc.vector.tensor_copy(out=cnt0_sb[:], in_=psum0[:])
            nc.vector.tensor_copy(out=cnt1_sb[:], in_=psum1[:])
            nc.sync.dma_start(
                out=count_part.ap()[0:N_SLOTS // 2].rearrange(
                    "(p l) -> p l", p=128),
                in_=cnt0_sb[:])
            nc.sync.dma_start(
                out=count_part.ap()[N_SLOTS // 2:N_SLOTS].rearrange(
                    "(p l) -> p l", p=128),
                in_=cnt1_sb[:])

            # ---- Phase 4: sum partials across cores; each core keeps its slice
            nc.gpsimd.collective_compute(
                "ReduceScatter",
                mybir.AluOpType.add,
                replica_groups=[list(range(NCORES))],
                ins=[count_part.ap()],
                outs=[count_rs.ap()],
            )

            # ---- Phase 5: out[s, :] = count[s] * rv_row ----
            cnt_sb_bf = ipool.tile([128, OUT_GROUPS], BF16)
            nc.sync.dma_start(
                out=cnt_sb_bf[:],
                in_=count_rs.ap().rearrange("(g p) -> p g", p=128))
            cnt_sb = ipool.tile([128, OUT_GROUPS], F32)
            nc.vector.tensor_copy(out=cnt_sb[:], in_=cnt_sb_bf[:])
            DCH = 8  # groups per output DMA chunk
            for g0 in range(0, OUT_GROUPS, DCH):
                ot = opool.tile([128, DCH * HIDDEN], F32)
                nc.vector.tensor_tensor(
                    out=ot[:].rearrange("p (g h) -> p g h", h=HIDDEN),
                    in0=rv_row[:].rearrange(
                        "p (o h) -> p o h", o=1).to_broadcast(
                        [128, DCH, HIDDEN]),
                    in1=cnt_sb[:, g0:g0 + DCH].rearrange(
                        "p (g o) -> p g o", o=1).to_broadcast(
                        [128, DCH, HIDDEN]),
                    op=mybir.AluOpType.mult)
                nc.sync.dma_start(
                    out=out.ap()[g0 * 128:(g0 + DCH) * 128, :].rearrange(
                        "(g p) h -> p g h", p=128),
                    in_=ot[:].rearrange("p (g h) -> p g h", h=HIDDEN))

    nc.compile()
    return nc


_NC = None


def _get_nc():
    global _NC
    if _NC is None:
        _NC = build_kernel()
    return _NC


def make_in_maps(batch_tails, batch_heads, W, b):
    tails = np.ascontiguousarray(batch_tails, dtype=np.int32)
    heads = np.ascontiguousarray(batch_heads, dtype=np.int32)
    W32 = np.ascontiguousarray(W, dtype=np.float32)
    b32 = np.ascontiguousarray(b, dtype=np.float32)
    iota_bf = np.tile(np.arange(256, dtype=np.float32).astype(
        ml_dtypes.bfloat16), (128, 1))
    ident = np.eye(128, dtype=np.float32)
    offrow = np.zeros(COLS, dtype=np.float32)
    nfull = (COLS // BLK) * BLK
    blk = np.arange(COLS) % BLK
    offrow[:nfull] = np.where(blk[:nfull] < PHI, 256.0 * blk[:nfull], 0.0)
    offpat = np.tile(offrow, (128, 1))
    in_maps = []
    for k in range(NCORES):
        sl = slice(k * FACTS_PER_CORE, (k + 1) * FACTS_PER_CORE)
        in_maps.append({
            "tails": tails[sl],
            "heads": heads[sl],
            "W": W32,
            "b": b32,
            "iota_bf": iota_bf,
            "ident": ident,
            "offpat": offpat,
        })
    return in_maps


def kernel(local_entity, batch_heads, batch_rels, batch_tails, batch_ids,
           fact_ids, W, b, **_unused):
    nc = _get_nc()
    in_maps = make_in_maps(batch_tails, batch_heads, W, b)
    res = run_bass_kernel_spmd(nc, in_maps, list(range(NCORES)))
    full = np.concatenate([res.results[k]["out"] for k in range(NCORES)],
                          axis=0)
    return full.reshape(BATCH, MAX_LOCAL_ENTITY, HIDDEN)


if __name__ == "__main__":
    rng = np.random.default_rng(0)
    n_slots = BATCH * MAX_LOCAL_ENTITY
    heads = rng.integers(0, n_slots, NUM_FACT).astype(np.int64)
    tails = rng.integers(0, n_slots, NUM_FACT).astype(np.int64)
    W = rng.standard_normal((HIDDEN, HIDDEN)).astype(np.float32) * 0.05
    b = rng.standard_normal(HIDDEN).astype(np.float32) * 0.05
    got = kernel(local_entity=None, batch_heads=heads, batch_rels=None,
                 batch_tails=tails, batch_ids=None, fact_ids=None, W=W, b=b)
    v = W.sum(axis=1) + b
    count = (np.bincount(tails, minlength=n_slots)
             + np.bincount(heads, minlength=n_slots)).astype(np.float32)
    want = np.maximum(count[:, None] * v[None, :], 0.0).reshape(
        BATCH, MAX_LOCAL_ENTITY, HIDDEN)
    err = np.abs(got - want).max()
    rel = err / max(np.abs(want).max(), 1e-12)
    print("max abs err:", err, "rel:", rel)
    assert rel < 1e-4, "MISMATCH"
    print("KERNEL OK")


# revision 35
# speedup vs baseline: 1.0173x; 1.0173x over previous
"""Trainium2 Bass kernel for nn_BackwardReasonModel (gnn_message_passing).

Math reduction: fact_rel is all-ones so every row of fact_val equals
v = W.sum(axis=1) + b.  The two scatter-adds therefore produce
agg[s, :] = count[s] * v where count[s] = #occurrences of slot s in
batch_tails ++ batch_heads, and relu(count * v) = count * relu(v) since
count >= 0.  The kernel computes the 65536-bin histogram of the 1M
indices on-device via one-hot matmuls on the TensorEngine (256x256 bin
factorization, accumulated in PSUM), ReduceScatters the per-core partial
histograms across the 8 cores, and each core then writes its 8192-slot
slice of the output as count ⊗ relu(v).

Sharding: facts are split 8 ways (65536 facts -> 131072 head+tail
indices per core); output slots are split 8 ways by the ReduceScatter.
"""

import numpy as np
import ml_dtypes

import concourse.mybir as mybir
import concourse.tile as tile
import concourse.bacc as bacc
from concourse import library_config
from concourse.bass_utils import run_bass_kernel_spmd

NCORES = 8
BATCH = 32
MAX_LOCAL_ENTITY = 2048
NUM_FACT = 524288
HIDDEN = 128
N_SLOTS = BATCH * MAX_LOCAL_ENTITY          # 65536
FACTS_PER_CORE = NUM_FACT // NCORES         # 65536
N_IDX = 2 * FACTS_PER_CORE                  # 131072 indices per core
COLS = N_IDX // 128                         # 1024 columns of 128 indices
SLOTS_PER_CORE = N_SLOTS // NCORES          # 8192
OUT_GROUPS = SLOTS_PER_CORE // 128          # 64

F32 = mybir.dt.float32
BF16 = mybir.dt.bfloat16
I32 = mybir.dt.int32

# column blocking: per block of BLK columns, the first PHI hi-one-hots and
# the first PLO lo-one-hots are built by Pool local_scatter ops, the next
# AHI hi-one-hots by the scalar engine (Abs + Relu), the rest plus the
# remaining lo-one-hots by the DVE is_equal.
BLK = 12
PHI = 7
AHI = 3
PLO = 4


def build_kernel():
    nc = bacc.Bacc("TRN2", target_bir_lowering=False, debug=False,
                   num_devices=NCORES)

    tails = nc.dram_tensor("tails", [FACTS_PER_CORE], I32, kind="ExternalInput")
    heads = nc.dram_tensor("heads", [FACTS_PER_CORE], I32, kind="ExternalInput")
    w_in = nc.dram_tensor("W", [HIDDEN, HIDDEN], F32, kind="ExternalInput")
    b_in = nc.dram_tensor("b", [HIDDEN], F32, kind="ExternalInput")
    out = nc.dram_tensor("out", [SLOTS_PER_CORE, HIDDEN], F32,
                         kind="ExternalOutput")
    count_part = nc.dram_tensor("count_part", [N_SLOTS], BF16)
    count_rs = nc.dram_tensor("count_rs", [SLOTS_PER_CORE], BF16)
    iota_in = nc.dram_tensor("iota_bf", [128, 256], BF16, kind="ExternalInput")
    ident_in = nc.dram_tensor("ident", [128, 128], F32, kind="ExternalInput")
    offpat_in = nc.dram_tensor("offpat", [128, COLS], F32,
                               kind="ExternalInput")

    NBLK = COLS // BLK          # 85 full blocks
    TAIL0 = NBLK * BLK          # 1020; tail columns go to DVE

    with tile.TileContext(nc) as tc:
        with (
            tc.tile_pool(name="const", bufs=1) as cpool,
            tc.tile_pool(name="idx", bufs=1) as ipool,
            tc.tile_pool(name="oh", bufs=12) as ohpool,
            tc.tile_pool(name="poh", bufs=6) as pohpool,
            tc.tile_pool(name="outp", bufs=8) as opool,
            tc.tile_pool(name="psum", bufs=1, space="PSUM") as ppool,
        ):
            nc.gpsimd.load_library(library_config.local_scatter)

            # ---- Phase 0: constants + rv_row = relu(W.sum(1) + b) as a row
            iota_bf = cpool.tile([128, 256], BF16)
            nc.sync.dma_start(out=iota_bf[:], in_=iota_in.ap())
            ident = cpool.tile([128, 128], F32)
            nc.sync.dma_start(out=ident[:], in_=ident_in.ap())
            w_t = cpool.tile([128, 128], F32)
            nc.sync.dma_start(out=w_t[:], in_=w_in.ap())
            b_t = cpool.tile([128, 1], F32)
            nc.sync.dma_start(out=b_t[:], in_=b_in.ap().rearrange("(p o) -> p o", o=1))
            v_col = cpool.tile([128, 1], F32)
            nc.vector.reduce_sum(v_col[:], w_t[:], axis=mybir.AxisListType.X)
            v2_col = cpool.tile([128, 1], F32)
            nc.vector.tensor_tensor(out=v2_col[:], in0=v_col[:], in1=b_t[:],
                                    op=mybir.AluOpType.add)
            v_row_ps = ppool.tile([128, 128], F32)
            nc.tensor.transpose(out=v_row_ps[:],
                                in_=v2_col[:].to_broadcast([128, 128]),
                                identity=ident[:])
            rv_row = cpool.tile([128, 128], F32)
            nc.vector.tensor_scalar_max(rv_row[:], v_row_ps[:], 0.0)

            # ---- Phase 1: load indices, split into hi/lo planes ----
            idx = ipool.tile([128, COLS], I32)
            nc.sync.dma_start(
                out=idx[:, 0:COLS // 2],
                in_=tails.ap().rearrange("(p c) -> p c", p=128))
            nc.sync.dma_start(
                out=idx[:, COLS // 2:COLS],
                in_=heads.ap().rearrange("(p c) -> p c", p=128))
            hi_i = ipool.tile([128, COLS], I32)
            nc.vector.tensor_scalar(hi_i[:], idx[:], 8, None,
                                    op0=mybir.AluOpType.logical_shift_right)
            lo_i = ipool.tile([128, COLS], I32)
            nc.vector.tensor_scalar(lo_i[:], idx[:], 255, None,
                                    op0=mybir.AluOpType.bitwise_and)
            hi_f = ipool.tile([128, COLS], F32)
            nc.vector.tensor_copy(out=hi_f[:], in_=hi_i[:])
            lo_f = ipool.tile([128, COLS], F32)
            nc.vector.tensor_copy(out=lo_f[:], in_=lo_i[:])

            # Pool local_scatter index pairs: per block, 8 int16 idxs
            # [hi0, hi1+256, .., hi6+1536, -1] (offsets make them unique)
            offp = ipool.tile([128, COLS], F32)
            nc.sync.dma_start(out=offp[:], in_=offpat_in.ap())
            hioff_f = ipool.tile([128, COLS], F32)
            nc.vector.tensor_tensor(out=hioff_f[:], in0=hi_f[:], in1=offp[:],
                                    op=mybir.AluOpType.add)
            pidx = ipool.tile([128, 8 * NBLK], mybir.dt.int16)
            nc.vector.memset(pidx[:], -1)
            nc.vector.tensor_copy(
                out=pidx[:].rearrange("p (g b) -> p g b", b=8)[:, :, 0:PHI],
                in_=hioff_f[:, 0:TAIL0].rearrange(
                    "p (g b) -> p g b", b=BLK)[:, :, 0:PHI])
            looff_f = ipool.tile([128, COLS], F32)
            nc.vector.tensor_tensor(out=looff_f[:], in0=lo_f[:], in1=offp[:],
                                    op=mybir.AluOpType.add)
            pidx_lo = ipool.tile([128, 8 * NBLK], mybir.dt.int16)
            nc.vector.memset(pidx_lo[:], -1)
            nc.vector.tensor_copy(
                out=pidx_lo[:].rearrange("p (g b) -> p g b", b=8)[:, :, 0:PLO],
                in_=looff_f[:, 0:TAIL0].rearrange(
                    "p (g b) -> p g b", b=BLK)[:, :, 0:PLO])
            ones8 = cpool.tile([128, 8], BF16)
            nc.vector.memset(ones8[:], 1.0)

            # ---- Phase 2: histogram via one-hot matmuls ----
            psum0 = ppool.tile([128, 256], F32)
            psum1 = ppool.tile([128, 256], F32)

            def pool_lo(c):
                return BLK <= c < TAIL0 and c % BLK < PLO

            pt = None
            plt = None
            lo_cache = {}
            hi_cache = {}
            for c in range(COLS):
                r = c % BLK
                in_blk = BLK <= c < TAIL0
                if in_blk and r == 0:
                    g = c // BLK
                    pt = pohpool.tile([128, 256 * PHI], BF16)
                    nc.gpsimd.local_scatter(
                        pt[:], ones8[:], pidx[:, 8 * g:8 * g + 8],
                        channels=128, num_elems=256 * PHI, num_idxs=8)
                    plt = pohpool.tile([128, 256 * PLO], BF16)
                    nc.gpsimd.local_scatter(
                        plt[:], ones8[:], pidx_lo[:, 8 * g:8 * g + 8],
                        channels=128, num_elems=256 * PLO, num_idxs=8)
                if in_blk and r < PHI:
                    st0 = pt[:, 256 * r:256 * r + 128]
                    st1 = pt[:, 256 * r + 128:256 * (r + 1)]
                elif in_blk and r < PHI + AHI:
                    # hi one-hot on the scalar engine:
                    # t = |iota - hi|; onehot = relu(1 - t)
                    t = ohpool.tile([128, 256], BF16)
                    nc.scalar.activation(t[:], iota_bf[:],
                                         mybir.ActivationFunctionType.Abs,
                                         bias=hi_f[:, c:c + 1], scale=-1.0)
                    hi_oh = ohpool.tile([128, 256], BF16)
                    nc.scalar.activation(hi_oh[:], t[:],
                                         mybir.ActivationFunctionType.Relu,
                                         bias=1.0, scale=-1.0)
                    st0, st1 = hi_oh[:, 0:128], hi_oh[:, 128:256]
                elif c in hi_cache:
                    st0, st1 = hi_cache.pop(c)
                else:
                    nxt = c + 1
                    nxt_dve = (nxt < COLS and not (
                        BLK <= nxt < TAIL0 and nxt % BLK < PHI + AHI))
                    if nxt_dve:
                        hi2 = ohpool.tile([128, 512], BF16)
                        nc.vector.tensor_scalar(
                            hi2[:, 0:256], iota_bf[:], hi_f[:, c:c + 1], None,
                            op0=mybir.AluOpType.is_equal)
                        nc.vector.tensor_scalar(
                            hi2[:, 256:512], iota_bf[:], hi_f[:, nxt:nxt + 1],
                            None, op0=mybir.AluOpType.is_equal)
                        hi_cache[nxt] = (hi2[:, 256:384], hi2[:, 384:512])
                        st0, st1 = hi2[:, 0:128], hi2[:, 128:256]
                    else:
                        hi_oh = ohpool.tile([128, 256], BF16)
                        nc.vector.tensor_scalar(hi_oh[:], iota_bf[:],
                                                hi_f[:, c:c + 1], None,
                                                op0=mybir.AluOpType.is_equal)
                        st0, st1 = hi_oh[:, 0:128], hi_oh[:, 128:256]
                if pool_lo(c):
                    lo_mv = plt[:, 256 * r:256 * (r + 1)]
                elif c in lo_cache:
                    lo_mv = lo_cache.pop(c)
                else:
                    # group up to 4 consecutive DVE-built lo one-hots in one
                    # tile to cut Tile bookkeeping on the vector engine
                    run = [c]
                    while (len(run) < 4 and run[-1] + 1 < COLS
                           and not pool_lo(run[-1] + 1)):
                        run.append(run[-1] + 1)
                    loq = ohpool.tile([128, 256 * len(run)], BF16)
                    for j, cc in enumerate(run):
                        nc.vector.tensor_scalar(
                            loq[:, 256 * j:256 * (j + 1)], iota_bf[:],
                            lo_f[:, cc:cc + 1], None,
                            op0=mybir.AluOpType.is_equal)
                        if j:
                            lo_cache[cc] = loq[:, 256 * j:256 * (j + 1)]
                    lo_mv = loq[:, 0:256]
                first = c == 0
                last = c == COLS - 1
                nc.tensor.matmul(out=psum0[:], lhsT=st0,
                                 rhs=lo_mv, start=first, stop=last)
                nc.tensor.matmul(out=psum1[:], lhsT=st1,
                                 rhs=lo_mv, start=first, stop=last)

            # ---- Phase 3: flatten per-core histogram to DRAM ----
            cnt0_sb = ipool.tile([128, 256], BF16)
            cnt1_sb = ipool.tile([128, 256], BF16)
            nc.vector.tensor_copy(out=cnt0_sb[:], in_=psum0[:])
            nc.vector.tensor_copy(out=cnt1_sb[:], in_=psum1[:])
            nc.sync.dma_start(
                out=count_part.ap()[0:N_SLOTS // 2].rearrange(
                    "(p l) -> p l", p=128),
                in_=cnt0_sb[:])
            nc.sync.dma_start(
                out=count_part.ap()[N_SLOTS // 2:N_SLOTS].rearrange(
                    "(p l) -> p l", p=128),
                in_=cnt1_sb[:])

            # ---- Phase 4: sum partials across cores; each core keeps its slice
            nc.gpsimd.collective_compute(
                "ReduceScatter",
                mybir.AluOpType.add,
                replica_groups=[list(range(NCORES))],
                ins=[count_part.ap()],
                outs=[count_rs.ap()],
            )

            # ---- Phase 5: out[s, :] = count[s] * rv_row ----
            cnt_sb_bf = ipool.tile([128, OUT_GROUPS], BF16)
            nc.sync.dma_start(
                out=cnt_sb_bf[:],
                in_=count_rs.ap().rearrange("(g p) -> p g", p=128))
            cnt_sb = ipool.tile([128, OUT_GROUPS], F32)
            nc.vector.tensor_copy(out=cnt_sb[:], in_=cnt_sb_bf[:])
            DCH = 8  # groups per output DMA chunk
            for g0 in range(0, OUT_GROUPS, DCH):
                ot = opool.tile([128, DCH * HIDDEN], F32)
                nc.vector.tensor_tensor(
                    out=ot[:].rearrange("p (g h) -> p g h", h=HIDDEN),
                    in0=rv_row[:].rearrange(
                        "p (o h) -> p o h", o=1).to_broadcast(
                        [128, DCH, HIDDEN]),
                    in1=cnt_sb[:, g0:g0 + DCH].rearrange(
                        "p (g o) -> p g o", o=1).to_broadcast(
                        [128, DCH, HIDDEN]),
                    op=mybir.AluOpType.mult)
                nc.sync.dma_start(
                    out=out.ap()[g0 * 128:(g0 + DCH) * 128, :].rearrange(
                        "(g p) h -> p g h", p=128),
                    in_=ot[:].rearrange("p (g h) -> p g h", h=HIDDEN))

    nc.compile()
    return nc


_NC = None


def _get_nc():
    global _NC
    if _NC is None:
        _NC = build_kernel()
    return _NC


def make_in_maps(batch_tails, batch_heads, W, b):
    tails = np.ascontiguousarray(batch_tails, dtype=np.int32)
    heads = np.ascontiguousarray(batch_heads, dtype=np.int32)
    W32 = np.ascontiguousarray(W, dtype=np.float32)
    b32 = np.ascontiguousarray(b, dtype=np.float32)
    iota_bf = np.tile(np.arange(256, dtype=np.float32).astype(
        ml_dtypes.bfloat16), (128, 1))
    ident = np.eye(128, dtype=np.float32)
    offrow = np.zeros(COLS, dtype=np.float32)
    nfull = (COLS // BLK) * BLK
    blk = np.arange(COLS) % BLK
    offrow[:nfull] = np.where(blk[:nfull] < PHI, 256.0 * blk[:nfull], 0.0)
    offpat = np.tile(offrow, (128, 1))
    in_maps = []
    for k in range(NCORES):
        sl = slice(k * FACTS_PER_CORE, (k + 1) * FACTS_PER_CORE)
        in_maps.append({
            "tails": tails[sl],
            "heads": heads[sl],
            "W": W32,
            "b": b32,
            "iota_bf": iota_bf,
            "ident": ident,
            "offpat": offpat,
        })
    return in_maps


def kernel(local_entity, batch_heads, batch_rels, batch_tails, batch_ids,
           fact_ids, W, b, **_unused):
    nc = _get_nc()
    in_maps = make_in_maps(batch_tails, batch_heads, W, b)
    res = run_bass_kernel_spmd(nc, in_maps, list(range(NCORES)))
    full = np.concatenate([res.results[k]["out"] for k in range(NCORES)],
                          axis=0)
    return full.reshape(BATCH, MAX_LOCAL_ENTITY, HIDDEN)


if __name__ == "__main__":
    rng = np.random.default_rng(0)
    n_slots = BATCH * MAX_LOCAL_ENTITY
    heads = rng.integers(0, n_slots, NUM_FACT).astype(np.int64)
    tails = rng.integers(0, n_slots, NUM_FACT).astype(np.int64)
    W = rng.standard_normal((HIDDEN, HIDDEN)).astype(np.float32) * 0.05
    b = rng.standard_normal(HIDDEN).astype(np.float32) * 0.05
    got = kernel(local_entity=None, batch_heads=heads, batch_rels=None,
                 batch_tails=tails, batch_ids=None, fact_ids=None, W=W, b=b)
    v = W.sum(axis=1) + b
    count = (np.bincount(tails, minlength=n_slots)
             + np.bincount(heads, minlength=n_slots)).astype(np.float32)
    want = np.maximum(count[:, None] * v[None, :], 0.0).reshape(
        BATCH, MAX_LOCAL_ENTITY, HIDDEN)
    err = np.abs(got - want).max()
    rel = err / max(np.abs(want).max(), 1e-12)
    print("max abs err:", err, "rel:", rel)
    assert rel < 1e-4, "MISMATCH"
    print("KERNEL OK")
